# revision 1
# baseline (speedup 1.0000x reference)
"""Causal single-head attention (b=4, n=2048, d=1024, fp32) on 8 TRN2 NeuronCores.

Sharding: each core handles (batch = c//2, output-feature half = c%2).
All 8 cores run one SPMD Bass module; per-core behavior differs only in data.

Per core (batch b, o-half h):
  - inputs (host-prepped, bf16): xT = x[b].T [1024,2048], wqT/wkT = W.T [1024,1024],
    wvT = W_v.T[:, h*512:(h+1)*512] [1024,512], causal masks [4,128,512].
  - projections (PE, bf16 -> fp32 PSUM -> bf16 SBUF):
      qT[o,q] = W_q x[b].T   (full),  kT[o,k] = W_k x[b].T  (full),
      v[k,o_h] = x[b] W_v.T half.
  - scores, transposed layout sT[k,q] = kT.T-contraction over o; softmax over k
    (partition axis): exp on ScalarE (no max subtraction - scaled scores are in
    [-2.6, 2.6]), causal zeroing by mask multiply, row sums via ones-matmul,
    out[q,o_h] = P.T-contraction over k, normalize by reciprocal sums.
  - causal skipping: strictly-upper k-chunks never computed (uniform across
    cores since every core sees all 2048 queries).
Output per core: [2048, 512] fp32 -> host writes out[b, :, h*512:(h+1)*512].
"""

import os
import sys

sys.path.insert(0, "/opt/trn_rl_repo")

import numpy as np
import ml_dtypes

BF16 = ml_dtypes.bfloat16

B, N, D = 4, 2048, 1024
NCORES = 8
P = 128          # partition / chunk size
OH = D // 2      # per-core output-feature half
QT = 512         # q tile width (moving-operand width)
NQT = N // QT    # 4 q tiles
NKC = N // P     # 16 k chunks
NDC = D // P     # 8 d chunks (projection contraction)
NOC = D // P     # 8 o chunks (score contraction)
SCALE = 1.0 / 32.0  # 1/sqrt(d_out)

_CACHE = {}


def _build_module():
    from concourse import bacc
    import concourse.tile as tile
    import concourse.mybir as mybir

    bf = mybir.dt.bfloat16
    f32 = mybir.dt.float32
    Exp = mybir.ActivationFunctionType.Exp

    nc = bacc.Bacc("TRN2", target_bir_lowering=False, debug=False, num_devices=NCORES)

    xT_d = nc.dram_tensor("xT", [D, N], bf, kind="ExternalInput")
    wqT_d = nc.dram_tensor("wqT", [D, D], bf, kind="ExternalInput")
    wkT_d = nc.dram_tensor("wkT", [D, D], bf, kind="ExternalInput")
    wvT_d = nc.dram_tensor("wvT", [D, OH], bf, kind="ExternalInput")
    mk_d = nc.dram_tensor("masks", [4, P, QT], bf, kind="ExternalInput")
    out_d = nc.dram_tensor("out", [N, OH], f32, kind="ExternalOutput")

    xT_r = xT_d.ap().rearrange("(dc p) n -> p dc n", p=P)
    wq_r = wqT_d.ap().rearrange("(dc p) o -> p dc o", p=P)
    wk_r = wkT_d.ap().rearrange("(dc p) o -> p dc o", p=P)
    wv_r = wvT_d.ap().rearrange("(dc p) o -> p dc o", p=P)
    mk_r = mk_d.ap().rearrange("j p q -> p j q")
    out_r = out_d.ap().rearrange("(s p) o -> p s o", p=P)

    with tile.TileContext(nc) as tc:
        with tc.tile_pool(name="pers", bufs=1) as pers:
            xT = pers.tile([P, NDC, N], bf, tag="xT")
            wq = pers.tile([P, NDC, D], bf, tag="wq")
            wk = pers.tile([P, NDC, D], bf, tag="wk")
            wv = pers.tile([P, NDC, OH], bf, tag="wv")
            qT = pers.tile([P, NOC, N], bf, tag="qT")
            kT = pers.tile([P, NOC, N], bf, tag="kT")
            v = pers.tile([P, NKC, OH], bf, tag="v")
            mks = pers.tile([P, 4, QT], bf, tag="masks")
            ones = pers.tile([P, 1], bf, tag="ones")

            for dc in range(NDC):
                nc.sync.dma_start(xT[:, dc, :], xT_r[:, dc, :])
                nc.sync.dma_start(wk[:, dc, :], wk_r[:, dc, :])
                nc.sync.dma_start(wq[:, dc, :], wq_r[:, dc, :])
                nc.sync.dma_start(wv[:, dc, :], wv_r[:, dc, :])
            nc.sync.dma_start(mks[:], mk_r)
            nc.vector.memset(ones[:], 1.0)

            # ---- projections ----
            with tc.tile_pool(name="psA", bufs=4, space="PSUM") as psA:
                # kT[o, k] += wkT[d, o].T @ xT[d, k]
                for kt in range(NQT):
                    for oc in range(NOC):
                        ps = psA.tile([P, QT], f32, tag="proj")
                        for dc in range(NDC):
                            nc.tensor.matmul(
                                ps,
                                wk[:, dc, oc * P : (oc + 1) * P],
                                xT[:, dc, kt * QT : (kt + 1) * QT],
                                start=(dc == 0),
                                stop=(dc == NDC - 1),
                            )
                        nc.vector.tensor_copy(kT[:, oc, kt * QT : (kt + 1) * QT], ps)
                # qT[o, q] += wqT[d, o].T @ xT[d, q]
                for qt in range(NQT):
                    for oc in range(NOC):
                        ps = psA.tile([P, QT], f32, tag="proj")
                        for dc in range(NDC):
                            nc.tensor.matmul(
                                ps,
                                wq[:, dc, oc * P : (oc + 1) * P],
                                xT[:, dc, qt * QT : (qt + 1) * QT],
                                start=(dc == 0),
                                stop=(dc == NDC - 1),
                            )
                        nc.vector.tensor_copy(qT[:, oc, qt * QT : (qt + 1) * QT], ps)
                # v[k, o_h] += xT[d, k].T @ wvT[d, o_h]
                for kc in range(NKC):
                    ps = psA.tile([P, OH], f32, tag="proj")
                    for dc in range(NDC):
                        nc.tensor.matmul(
                            ps,
                            xT[:, dc, kc * P : (kc + 1) * P],
                            wv[:, dc, :],
                            start=(dc == 0),
                            stop=(dc == NDC - 1),
                        )
                    nc.vector.tensor_copy(v[:, kc, :], ps)

            # ---- attention ----
            with (
                tc.tile_pool(name="stps", bufs=3, space="PSUM") as stps,
                tc.tile_pool(name="avps", bufs=2, space="PSUM") as avps,
                tc.tile_pool(name="smps", bufs=2, space="PSUM") as smps,
                tc.tile_pool(name="pTp", bufs=2) as pTp,
                tc.tile_pool(name="outst", bufs=4) as outst,
                tc.tile_pool(name="rcpp", bufs=4) as rcpp,
            ):
                for t in range(NQT):
                    nkc = 4 * t + 4  # kept k chunks for this q tile
                    sheet = pTp.tile([P, NKC, QT], bf, tag="sheet")
                    for c in range(nkc):
                        ps = stps.tile([P, QT], f32, tag="st")
                        for oc in range(NOC):
                            nc.tensor.matmul(
                                ps,
                                kT[:, oc, c * P : (c + 1) * P],
                                qT[:, oc, t * QT : (t + 1) * QT],
                                start=(oc == 0),
                                stop=(oc == NOC - 1),
                            )
                        nc.scalar.activation(sheet[:, c, :], ps, Exp, bias=0.0, scale=SCALE)
                        if c >= 4 * t:
                            j = c - 4 * t
                            nc.vector.tensor_mul(
                                sheet[:, c, :], sheet[:, c, :], mks[:, j, :]
                            )
                    for jq in range(4):
                        s = 4 * t + jq  # global q subtile
                        av = avps.tile([P, OH], f32, tag="av")
                        sm = smps.tile([P, 1], f32, tag="sm")
                        for c in range(s + 1):
                            psl = sheet[:, c, jq * P : (jq + 1) * P]
                            nc.tensor.matmul(
                                av, psl, v[:, c, :], start=(c == 0), stop=(c == s)
                            )
                            nc.tensor.matmul(
                                sm, psl, ones[:], start=(c == 0), stop=(c == s)
                            )
                        r = rcpp.tile([P, 1], f32, tag="rcp")
                        nc.vector.reciprocal(r[:], sm)
                        ot = outst.tile([P, OH], f32, tag="out")
                        nc.vector.tensor_scalar_mul(ot[:], av, r[:])
                        nc.sync.dma_start(out_r[:, s, :], ot[:])

    nc.compile()
    return nc


def _masks_np():
    k = np.arange(P)[:, None]
    q = np.arange(QT)[None, :]
    return np.stack([(k + P * j <= q) for j in range(4)]).astype(BF16)


def get_module():
    if "nc" not in _CACHE:
        _CACHE["nc"] = _build_module()
    return _CACHE["nc"]


def make_in_maps(x, W_q, W_k, W_v):
    xT = np.ascontiguousarray(np.asarray(x, dtype=np.float32).transpose(0, 2, 1)).astype(BF16)
    wqT = np.ascontiguousarray(np.asarray(W_q, dtype=np.float32).T).astype(BF16)
    wkT = np.ascontiguousarray(np.asarray(W_k, dtype=np.float32).T).astype(BF16)
    wvT = np.ascontiguousarray(np.asarray(W_v, dtype=np.float32).T).astype(BF16)
    masks = _masks_np()
    in_maps = []
    for c in range(NCORES):
        b, h = c // 2, c % 2
        in_maps.append(
            {
                "xT": xT[b],
                "wqT": wqT,
                "wkT": wkT,
                "wvT": np.ascontiguousarray(wvT[:, h * OH : (h + 1) * OH]),
                "masks": masks,
            }
        )
    return in_maps


def kernel(x, W_q, W_k, W_v):
    from concourse.bass_utils import run_bass_kernel_spmd

    nc = get_module()
    in_maps = make_in_maps(x, W_q, W_k, W_v)
    res = run_bass_kernel_spmd(
        nc,
        in_maps,
        list(range(NCORES)),
        trace=bool(int(os.environ.get("KERNEL_TRACE", "0"))),
    )
    _CACHE["last_result"] = res
    out = np.empty((B, N, D), dtype=np.float32)
    for c in range(NCORES):
        b, h = c // 2, c % 2
        out[b, :, h * OH : (h + 1) * OH] = res.results[c]["out"]
    return out


# revision 2
# speedup vs baseline: 1.0352x; 1.0352x over previous
"""Causal single-head attention (b=4, n=2048, d=1024, fp32) on 8 TRN2 NeuronCores.

Sharding: each core handles (batch = c//2, output-feature half = c%2).
All 8 cores run one SPMD Bass module; per-core behavior differs only in data.

Per core (batch b, o-half h):
  - inputs (host-prepped, bf16): xT = x[b].T [1024,2048], wqT/wkT = W.T [1024,1024],
    wvT = W_v.T[:, h*512:(h+1)*512] [1024,512], causal masks [4,128,512].
  - projections (PE, bf16 -> fp32 PSUM -> bf16 SBUF):
      qT[o,q] = W_q x[b].T   (full),  kT[o,k] = W_k x[b].T  (full),
      v[k,o_h] = x[b] W_v.T half.
  - scores, transposed layout sT[k,q] = kT.T-contraction over o; softmax over k
    (partition axis): exp on ScalarE (no max subtraction - scaled scores are in
    [-2.6, 2.6]), causal zeroing by mask multiply, row sums via ones-matmul,
    out[q,o_h] = P.T-contraction over k, normalize by reciprocal sums.
  - causal skipping: strictly-upper k-chunks never computed (uniform across
    cores since every core sees all 2048 queries).
Output per core: [2048, 512] fp32 -> host writes out[b, :, h*512:(h+1)*512].
"""

import os
import sys

sys.path.insert(0, "/opt/trn_rl_repo")

import numpy as np
import ml_dtypes

BF16 = ml_dtypes.bfloat16

B, N, D = 4, 2048, 1024
NCORES = 8
P = 128          # partition / chunk size
OH = D // 2      # per-core output-feature half
QT = 512         # q tile width (moving-operand width)
NQT = N // QT    # 4 q tiles
NKC = N // P     # 16 k chunks
NDC = D // P     # 8 d chunks (projection contraction)
NOC = D // P     # 8 o chunks (score contraction)
SCALE = 1.0 / 32.0  # 1/sqrt(d_out)

_CACHE = {}


def _build_module():
    from concourse import bacc
    import concourse.tile as tile
    import concourse.mybir as mybir

    bf = mybir.dt.bfloat16
    f32 = mybir.dt.float32
    Exp = mybir.ActivationFunctionType.Exp

    nc = bacc.Bacc("TRN2", target_bir_lowering=False, debug=False, num_devices=NCORES)

    xT_d = nc.dram_tensor("xT", [D, N], bf, kind="ExternalInput")
    wqT_d = nc.dram_tensor("wqT", [D, D], bf, kind="ExternalInput")
    wkT_d = nc.dram_tensor("wkT", [D, D], bf, kind="ExternalInput")
    wvT_d = nc.dram_tensor("wvT", [D, OH], bf, kind="ExternalInput")
    mk_d = nc.dram_tensor("masks", [4, P, QT], bf, kind="ExternalInput")
    out_d = nc.dram_tensor("out", [N, OH], f32, kind="ExternalOutput")

    xT_r = xT_d.ap().rearrange("(dc p) n -> p dc n", p=P)
    wq_r = wqT_d.ap().rearrange("(dc p) o -> p dc o", p=P)
    wk_r = wkT_d.ap().rearrange("(dc p) o -> p dc o", p=P)
    wv_r = wvT_d.ap().rearrange("(dc p) o -> p dc o", p=P)
    mk_r = mk_d.ap().rearrange("j p q -> p j q")
    out_r = out_d.ap().rearrange("(s p) o -> p s o", p=P)

    with tile.TileContext(nc) as tc:
        with tc.tile_pool(name="pers", bufs=1) as pers:
            xT = pers.tile([P, NDC, N], bf, tag="xT")
            wq = pers.tile([P, NDC, D], bf, tag="wq")
            wk = pers.tile([P, NDC, D], bf, tag="wk")
            wv = pers.tile([P, NDC, OH], bf, tag="wv")
            qT = pers.tile([P, NOC, N], bf, tag="qT")
            kT = pers.tile([P, NOC, N], bf, tag="kT")
            v = pers.tile([P, NKC, OH], bf, tag="v")
            mks = pers.tile([P, 4, QT], bf, tag="masks")
            ones = pers.tile([P, 1], bf, tag="ones")

            # Issue order = consumption order: K-proj needs wk + xT[kt=0] first.
            # Split transfers ~256KB so they spread across the HW-DGE queues.
            for dc in range(NDC):
                nc.sync.dma_start(wk[:, dc, :D // 2], wk_r[:, dc, :D // 2])
                nc.sync.dma_start(wk[:, dc, D // 2 :], wk_r[:, dc, D // 2 :])
            for kt in range(NQT):
                for dc in range(NDC):
                    sl = slice(kt * QT, (kt + 1) * QT)
                    nc.sync.dma_start(xT[:, dc, sl], xT_r[:, dc, sl])
            for dc in range(NDC):
                nc.sync.dma_start(wq[:, dc, :D // 2], wq_r[:, dc, :D // 2])
                nc.sync.dma_start(wq[:, dc, D // 2 :], wq_r[:, dc, D // 2 :])
            for dc in range(NDC):
                nc.sync.dma_start(wv[:, dc, :], wv_r[:, dc, :])
            nc.sync.dma_start(mks[:], mk_r)
            nc.vector.memset(ones[:], 1.0)

            # ---- projections ----
            with tc.tile_pool(name="psA", bufs=4, space="PSUM") as psA:
                # kT[o, k] += wkT[d, o].T @ xT[d, k]
                for kt in range(NQT):
                    for oc in range(NOC):
                        ps = psA.tile([P, QT], f32, tag="proj")
                        for dc in range(NDC):
                            nc.tensor.matmul(
                                ps,
                                wk[:, dc, oc * P : (oc + 1) * P],
                                xT[:, dc, kt * QT : (kt + 1) * QT],
                                start=(dc == 0),
                                stop=(dc == NDC - 1),
                            )
                        nc.vector.tensor_copy(kT[:, oc, kt * QT : (kt + 1) * QT], ps)
                # qT[o, q] += wqT[d, o].T @ xT[d, q]
                for qt in range(NQT):
                    for oc in range(NOC):
                        ps = psA.tile([P, QT], f32, tag="proj")
                        for dc in range(NDC):
                            nc.tensor.matmul(
                                ps,
                                wq[:, dc, oc * P : (oc + 1) * P],
                                xT[:, dc, qt * QT : (qt + 1) * QT],
                                start=(dc == 0),
                                stop=(dc == NDC - 1),
                            )
                        nc.vector.tensor_copy(qT[:, oc, qt * QT : (qt + 1) * QT], ps)
                # v[k, o_h] += xT[d, k].T @ wvT[d, o_h]
                for kc in range(NKC):
                    ps = psA.tile([P, OH], f32, tag="proj")
                    for dc in range(NDC):
                        nc.tensor.matmul(
                            ps,
                            xT[:, dc, kc * P : (kc + 1) * P],
                            wv[:, dc, :],
                            start=(dc == 0),
                            stop=(dc == NDC - 1),
                        )
                    nc.vector.tensor_copy(v[:, kc, :], ps)

            # ---- attention ----
            with (
                tc.tile_pool(name="stps", bufs=3, space="PSUM") as stps,
                tc.tile_pool(name="avps", bufs=2, space="PSUM") as avps,
                tc.tile_pool(name="smps", bufs=2, space="PSUM") as smps,
                tc.tile_pool(name="pTp", bufs=2) as pTp,
                tc.tile_pool(name="outst", bufs=4) as outst,
                tc.tile_pool(name="rcpp", bufs=4) as rcpp,
            ):
                for t in range(NQT):
                    nkc = 4 * t + 4  # kept k chunks for this q tile
                    sheet = pTp.tile([P, NKC, QT], bf, tag="sheet")
                    for c in range(nkc):
                        ps = stps.tile([P, QT], f32, tag="st")
                        for oc in range(NOC):
                            nc.tensor.matmul(
                                ps,
                                kT[:, oc, c * P : (c + 1) * P],
                                qT[:, oc, t * QT : (t + 1) * QT],
                                start=(oc == 0),
                                stop=(oc == NOC - 1),
                            )
                        nc.scalar.activation(sheet[:, c, :], ps, Exp, bias=0.0, scale=SCALE)
                        if c >= 4 * t:
                            j = c - 4 * t
                            nc.vector.tensor_mul(
                                sheet[:, c, :], sheet[:, c, :], mks[:, j, :]
                            )
                    for jq in range(4):
                        s = 4 * t + jq  # global q subtile
                        av = avps.tile([P, OH], f32, tag="av")
                        sm = smps.tile([P, 1], f32, tag="sm")
                        for c in range(s + 1):
                            psl = sheet[:, c, jq * P : (jq + 1) * P]
                            nc.tensor.matmul(
                                av, psl, v[:, c, :], start=(c == 0), stop=(c == s)
                            )
                            nc.tensor.matmul(
                                sm, psl, ones[:], start=(c == 0), stop=(c == s)
                            )
                        r = rcpp.tile([P, 1], f32, tag="rcp")
                        nc.vector.reciprocal(r[:], sm)
                        ot = outst.tile([P, OH], f32, tag="out")
                        nc.vector.tensor_scalar_mul(ot[:], av, r[:])
                        nc.sync.dma_start(out_r[:, s, :], ot[:])

    nc.compile()
    return nc


def _masks_np():
    k = np.arange(P)[:, None]
    q = np.arange(QT)[None, :]
    return np.stack([(k + P * j <= q) for j in range(4)]).astype(BF16)


def get_module():
    if "nc" not in _CACHE:
        _CACHE["nc"] = _build_module()
    return _CACHE["nc"]


def make_in_maps(x, W_q, W_k, W_v):
    xT = np.ascontiguousarray(np.asarray(x, dtype=np.float32).transpose(0, 2, 1)).astype(BF16)
    wqT = np.ascontiguousarray(np.asarray(W_q, dtype=np.float32).T).astype(BF16)
    wkT = np.ascontiguousarray(np.asarray(W_k, dtype=np.float32).T).astype(BF16)
    wvT = np.ascontiguousarray(np.asarray(W_v, dtype=np.float32).T).astype(BF16)
    masks = _masks_np()
    in_maps = []
    for c in range(NCORES):
        b, h = c // 2, c % 2
        in_maps.append(
            {
                "xT": xT[b],
                "wqT": wqT,
                "wkT": wkT,
                "wvT": np.ascontiguousarray(wvT[:, h * OH : (h + 1) * OH]),
                "masks": masks,
            }
        )
    return in_maps


def kernel(x, W_q, W_k, W_v):
    from concourse.bass_utils import run_bass_kernel_spmd

    nc = get_module()
    in_maps = make_in_maps(x, W_q, W_k, W_v)
    res = run_bass_kernel_spmd(
        nc,
        in_maps,
        list(range(NCORES)),
        trace=bool(int(os.environ.get("KERNEL_TRACE", "0"))),
    )
    _CACHE["last_result"] = res
    out = np.empty((B, N, D), dtype=np.float32)
    for c in range(NCORES):
        b, h = c // 2, c % 2
        out[b, :, h * OH : (h + 1) * OH] = res.results[c]["out"]
    return out


# revision 6
# speedup vs baseline: 1.1640x; 1.1244x over previous
"""Causal single-head attention (b=4, n=2048, d=1024, fp32) on 8 TRN2 NeuronCores.

Sharding v2 — uniform padded zig-zag q-split. Core c = (batch c//2, role c%2).
Role 0 owns query tiles (0, 3) of its batch, role 1 owns (1, 2); every core
produces out rows for its own 1024 queries with the FULL 1024 features.

The SPMD program is identical on all cores; the role only changes host-side
data: which columns land in xTq (own queries), the causal masks, and where
host scatters the output rows. Causal work is padded to the per-slot envelope
(slot0: 8 k-chunks, slot1: 16; AV per subtile j: 5+j / 13+j chunks) so both
roles run the same instruction stream; mask data zeroes the padding.

Per core pipeline (all matmuls bf16 -> fp32 PSUM):
  kT[o,k] = W_k x.T (full 2048 k), qT[o,q] = W_q xq.T (own 1024 q),
  v[k,o] = x W_v.T (full o);  sT[k,q] = scores (contraction over o);
  P = exp(sT/32) * mask (no max subtraction; scaled scores are in [-2.6, 2.6]);
  row sums l[q] via ones-matmul; out[q,o] = (P.T-contraction) / l.
"""

import os
import sys

sys.path.insert(0, "/opt/trn_rl_repo")

import numpy as np
import ml_dtypes

BF16 = ml_dtypes.bfloat16

B, N, D = 4, 2048, 1024
NCORES = 8
P = 128
QT = 512
NQT = N // QT      # 4 orig q tiles
NKC = N // P       # 16 k chunks
NDC = D // P       # 8 d chunks
NOC = D // P       # 8 o chunks
NQ_OWN = 1024      # own queries per core
SCALE = 1.0 / 32.0

ROLE_TILES = {0: (0, 3), 1: (1, 2)}
SLOT_NKC = (8, 16)                      # S^T envelope chunks per slot
AV_ENV = ((5, 6, 7, 8), (13, 14, 15, 16))  # AV envelope per (slot, subtile)

_CACHE = {}


def _build_module():
    from concourse import bacc
    import concourse.tile as tile
    import concourse.mybir as mybir

    bf = mybir.dt.bfloat16
    f32 = mybir.dt.float32
    Exp = mybir.ActivationFunctionType.Exp

    nc = bacc.Bacc("TRN2", target_bir_lowering=False, debug=False, num_devices=NCORES)

    xT_d = nc.dram_tensor("xT", [D, N], bf, kind="ExternalInput")
    xq_d = nc.dram_tensor("xTq", [D, NQ_OWN], bf, kind="ExternalInput")
    wqT_d = nc.dram_tensor("wqT", [D, D], bf, kind="ExternalInput")
    wkT_d = nc.dram_tensor("wkT", [D, D], bf, kind="ExternalInput")
    wvT_d = nc.dram_tensor("wvT", [D, D], bf, kind="ExternalInput")
    mk_d = nc.dram_tensor("masks", [24, P, QT], bf, kind="ExternalInput")
    out_d = nc.dram_tensor("out", [NQ_OWN, D], f32, kind="ExternalOutput")

    xT_r = xT_d.ap().rearrange("(dc p) n -> p dc n", p=P)
    xq_r = xq_d.ap().rearrange("(dc p) n -> p dc n", p=P)
    wq_r = wqT_d.ap().rearrange("(dc p) o -> p dc o", p=P)
    wk_r = wkT_d.ap().rearrange("(dc p) o -> p dc o", p=P)
    wv_r = wvT_d.ap().rearrange("(dc p) o -> p dc o", p=P)
    mk_r = mk_d.ap().rearrange("j p q -> p j q")
    out_r = out_d.ap().rearrange("(s p) o -> p s o", p=P)

    with tile.TileContext(nc) as tc:
        with tc.tile_pool(name="pers", bufs=1) as pers:
            qT = pers.tile([P, NOC, NQ_OWN], bf, tag="qT")
            kT = pers.tile([P, NOC, N], bf, tag="kT")
            v = pers.tile([P, NKC, D], bf, tag="v")
            mks = pers.tile([P, 24, QT], bf, tag="masks")
            ones = pers.tile([P, 1], bf, tag="ones")

            nc.vector.memset(ones[:], 1.0)

            # PE pre-warm while the first DMAs land (HAM ramp).
            with tc.tile_pool(name="warm", bufs=1, space="PSUM") as warmps:
                wsrc = pers.tile([P, QT], bf, tag="wsrc")
                nc.vector.memset(wsrc[:], 0.0)
                wps = warmps.tile([P, QT], f32, tag="warm")
                for _ in range(10):
                    nc.tensor.matmul(wps, wsrc[:, :P], wsrc[:], start=True, stop=True)

            # ---- projections (K, then Q, then V) ----
            with (
                tc.tile_pool(name="wp", bufs=1) as wp,
                tc.tile_pool(name="xsp", bufs=2) as xsp,
                tc.tile_pool(name="psA", bufs=4, space="PSUM") as psA,
            ):
                wk = wp.tile([P, NDC, D], bf, tag="wk")
                wq = wp.tile([P, NDC, D], bf, tag="wq")
                wv = wp.tile([P, NDC, D], bf, tag="wv")
                xts = []
                for kt in range(NQT):
                    xts.append(xsp.tile([P, NDC, QT], bf, tag="xs", name=f"xk{kt}"))
                # DMA issue order = consumption order.
                for dc in range(NDC):
                    nc.sync.dma_start(wk[:, dc, :], wk_r[:, dc, :])
                    nc.sync.dma_start(xts[0][:, dc, :], xT_r[:, dc, :QT])
                for kt in range(1, NQT):
                    for dc in range(NDC):
                        sl = slice(kt * QT, (kt + 1) * QT)
                        nc.sync.dma_start(xts[kt][:, dc, :], xT_r[:, dc, sl])
                for dc in range(NDC):
                    nc.sync.dma_start(wq[:, dc, :], wq_r[:, dc, :])
                for dc in range(NDC):
                    nc.sync.dma_start(wv[:, dc, :], wv_r[:, dc, :])
                nc.sync.dma_start(mks[:], mk_r)

                # K projection: kT[o, k] (full 2048 k)
                for kt in range(NQT):
                    for oc in range(NOC):
                        ps = psA.tile([P, QT], f32, tag="proj")
                        for dc in range(NDC):
                            nc.tensor.matmul(
                                ps,
                                wk[:, dc, oc * P : (oc + 1) * P],
                                xts[kt][:, dc, :],
                                start=(dc == 0),
                                stop=(dc == NDC - 1),
                            )
                        nc.vector.tensor_copy(kT[:, oc, kt * QT : (kt + 1) * QT], ps)

                # Q projection: qT[o, q] (own 1024 q)
                for qt in range(2):
                    xqt = xsp.tile([P, NDC, QT], bf, tag="xs", name=f"xq{qt}")
                    for dc in range(NDC):
                        sl = slice(qt * QT, (qt + 1) * QT)
                        nc.sync.dma_start(xqt[:, dc, :], xq_r[:, dc, sl])
                    for oc in range(NOC):
                        ps = psA.tile([P, QT], f32, tag="proj")
                        for dc in range(NDC):
                            nc.tensor.matmul(
                                ps,
                                wq[:, dc, oc * P : (oc + 1) * P],
                                xqt[:, dc, :],
                                start=(dc == 0),
                                stop=(dc == NDC - 1),
                            )
                        nc.vector.tensor_copy(qT[:, oc, qt * QT : (qt + 1) * QT], ps)

                # V projection: v[k, o] (full o)
                for kt in range(NQT):
                    xvt = xsp.tile([P, NDC, QT], bf, tag="xs", name=f"xv{kt}")
                    for dc in range(NDC):
                        sl = slice(kt * QT, (kt + 1) * QT)
                        nc.sync.dma_start(xvt[:, dc, :], xT_r[:, dc, sl])
                    for kl in range(4):
                        kc = kt * 4 + kl
                        for oh in range(2):
                            ps = psA.tile([P, QT], f32, tag="proj")
                            for dc in range(NDC):
                                nc.tensor.matmul(
                                    ps,
                                    xvt[:, dc, kl * P : (kl + 1) * P],
                                    wv[:, dc, oh * QT : (oh + 1) * QT],
                                    start=(dc == 0),
                                    stop=(dc == NDC - 1),
                                )
                            nc.vector.tensor_copy(
                                v[:, kc, oh * QT : (oh + 1) * QT], ps
                            )

            # ---- attention ----
            with (
                tc.tile_pool(name="stps", bufs=2, space="PSUM") as stps,
                tc.tile_pool(name="avps", bufs=2, space="PSUM") as avps,
                tc.tile_pool(name="smps", bufs=2, space="PSUM") as smps,
                tc.tile_pool(name="pTp", bufs=2) as pTp,
                tc.tile_pool(name="outst", bufs=4) as outst,
                tc.tile_pool(name="rcpp", bufs=4) as rcpp,
            ):
                for slot in range(2):
                    nk = SLOT_NKC[slot]
                    sheet = pTp.tile([P, NKC, QT], bf, tag="sheet")
                    for c in range(nk):
                        ps = stps.tile([P, QT], f32, tag="st")
                        for oc in range(NOC):
                            nc.tensor.matmul(
                                ps,
                                kT[:, oc, c * P : (c + 1) * P],
                                qT[:, oc, slot * QT : (slot + 1) * QT],
                                start=(oc == 0),
                                stop=(oc == NOC - 1),
                            )
                        nc.scalar.activation(
                            sheet[:, c, :], ps, Exp, bias=0.0, scale=SCALE
                        )
                        m = slot * 8 + c
                        nc.vector.tensor_mul(
                            sheet[:, c, :], sheet[:, c, :], mks[:, m, :]
                        )
                    for j in range(4):
                        e = AV_ENV[slot][j]
                        av = avps.tile([P, 2, QT], f32, tag="av")
                        sm = smps.tile([P, 1], f32, tag="sm")
                        for c in range(e):
                            psl = sheet[:, c, j * P : (j + 1) * P]
                            nc.tensor.matmul(
                                av[:, 0, :], psl, v[:, c, :QT],
                                start=(c == 0), stop=(c == e - 1),
                            )
                            nc.tensor.matmul(
                                av[:, 1, :], psl, v[:, c, QT:],
                                start=(c == 0), stop=(c == e - 1),
                            )
                            nc.tensor.matmul(
                                sm, psl, ones[:], start=(c == 0), stop=(c == e - 1)
                            )
                        r = rcpp.tile([P, 1], f32, tag="rcp")
                        nc.vector.reciprocal(r[:], sm)
                        ot = outst.tile([P, D], f32, tag="out")
                        nc.vector.tensor_scalar_mul(ot[:, :QT], av[:, 0, :], r[:])
                        nc.vector.tensor_scalar_mul(ot[:, QT:], av[:, 1, :], r[:])
                        nc.sync.dma_start(out_r[:, slot * 4 + j, :], ot[:])

    nc.compile()
    return nc


def _masks_np(role):
    g0, g1 = ROLE_TILES[role]
    k = np.arange(P)[:, None]
    q = np.arange(QT)[None, :]
    ms = []
    for c in range(8):
        ms.append(128 * c + k <= 512 * g0 + q)
    for c in range(16):
        ms.append(128 * c + k <= 512 * g1 + q)
    return np.stack(ms).astype(BF16)


def get_module():
    if "nc" not in _CACHE:
        _CACHE["nc"] = _build_module()
    return _CACHE["nc"]


def make_in_maps(x, W_q, W_k, W_v):
    xT = np.ascontiguousarray(
        np.asarray(x, dtype=np.float32).transpose(0, 2, 1)
    ).astype(BF16)
    wqT = np.ascontiguousarray(np.asarray(W_q, dtype=np.float32).T).astype(BF16)
    wkT = np.ascontiguousarray(np.asarray(W_k, dtype=np.float32).T).astype(BF16)
    wvT = np.ascontiguousarray(np.asarray(W_v, dtype=np.float32).T).astype(BF16)
    masks = [_masks_np(r) for r in range(2)]
    in_maps = []
    for c in range(NCORES):
        b, r = c // 2, c % 2
        g0, g1 = ROLE_TILES[r]
        xq = np.concatenate(
            [xT[b][:, g0 * QT : (g0 + 1) * QT], xT[b][:, g1 * QT : (g1 + 1) * QT]],
            axis=1,
        )
        in_maps.append(
            {
                "xT": xT[b],
                "xTq": np.ascontiguousarray(xq),
                "wqT": wqT,
                "wkT": wkT,
                "wvT": wvT,
                "masks": masks[r],
            }
        )
    return in_maps


def kernel(x, W_q, W_k, W_v):
    from concourse.bass_utils import run_bass_kernel_spmd

    nc = get_module()
    in_maps = make_in_maps(x, W_q, W_k, W_v)
    res = run_bass_kernel_spmd(
        nc,
        in_maps,
        list(range(NCORES)),
        trace=bool(int(os.environ.get("KERNEL_TRACE", "0"))),
    )
    _CACHE["last_result"] = res
    out = np.empty((B, N, D), dtype=np.float32)
    for c in range(NCORES):
        b, r = c // 2, c % 2
        g0, g1 = ROLE_TILES[r]
        res_out = res.results[c]["out"]
        out[b, g0 * QT : (g0 + 1) * QT, :] = res_out[:QT]
        out[b, g1 * QT : (g1 + 1) * QT, :] = res_out[QT:]
    return out


# revision 12
# speedup vs baseline: 1.1938x; 1.0256x over previous
"""Causal single-head attention (b=4, n=2048, d=1024, fp32) on 8 TRN2 NeuronCores.

Sharding v2 — uniform padded zig-zag q-split. Core c = (batch c//2, role c%2).
Each role owns 8 of the 16 query subtiles of its batch (zig-zag interleaved,
see ROLE_SUBTILES); every core produces out rows for its own 1024 queries
with the FULL 1024 features.

The SPMD program is identical on all cores; the role only changes host-side
data: which columns land in xTq (own queries), the causal masks, and where
host scatters the output rows. Causal work is padded to the per-slot envelope
(slot0: 8 k-chunks, slot1: 16; AV per subtile j: 5+j / 13+j chunks) so both
roles run the same instruction stream; mask data zeroes the padding.

Per core pipeline (all matmuls bf16 -> fp32 PSUM):
  kT[o,k] = W_k x.T (full 2048 k), qT[o,q] = W_q xq.T (own 1024 q),
  v[k,o] = x W_v.T (full o);  sT[k,q] = scores (contraction over o);
  P = exp(sT/32) * mask (no max subtraction; scaled scores are in [-2.6, 2.6]);
  row sums l[q] via ones-matmul; out[q,o] = (P.T-contraction) / l.
"""

import os
import sys

if os.path.isdir("/opt/trn_rl_repo") and "/opt/trn_rl_repo" not in sys.path:
    sys.path.insert(0, "/opt/trn_rl_repo")

import numpy as np
import ml_dtypes

BF16 = ml_dtypes.bfloat16

B, N, D = 4, 2048, 1024
NCORES = 8
P = 128
QT = 512
NQT = N // QT      # 4 orig q tiles
NKC = N // P       # 16 k chunks
NDC = D // P       # 8 d chunks
NOC = D // P       # 8 o chunks
NQ_OWN = 1024      # own queries per core
SCALE = 1.0 / 32.0

# Zig-zag assignment of the 16 query subtiles (128 rows each) to the two
# roles, chosen so the elementwise-max envelope across roles is minimal:
# slot0 = own subtiles drawn from {0..7}, slot1 from {8..15}.
ROLE_SUBTILES = {
    0: (0, 3, 4, 7, 8, 11, 12, 15),
    1: (1, 2, 5, 6, 9, 10, 13, 14),
}
SLOT_NKC = (8, 16)                  # S^T envelope chunks per slot
AV_ENV = ((2, 4, 6, 8), (10, 12, 14, 16))  # AV envelope per (slot, position)

_CACHE = {}


def _build_module():
    from concourse import bacc
    import concourse.tile as tile
    import concourse.mybir as mybir

    bf = mybir.dt.bfloat16
    f32 = mybir.dt.float32
    Exp = mybir.ActivationFunctionType.Exp

    nc = bacc.Bacc("TRN2", target_bir_lowering=False, debug=False, num_devices=NCORES)

    xT_d = nc.dram_tensor("xT", [D, N], bf, kind="ExternalInput")
    xq_d = nc.dram_tensor("xTq", [D, NQ_OWN], bf, kind="ExternalInput")
    wqT_d = nc.dram_tensor("wqT", [D, D], bf, kind="ExternalInput")
    wkT_d = nc.dram_tensor("wkT", [D, D], bf, kind="ExternalInput")
    wvT_d = nc.dram_tensor("wvT", [D, D], bf, kind="ExternalInput")
    mk_d = nc.dram_tensor("masks", [24, P, QT], bf, kind="ExternalInput")
    out_d = nc.dram_tensor("out", [NQ_OWN, D], f32, kind="ExternalOutput")

    xT_r = xT_d.ap().rearrange("(dc p) n -> p dc n", p=P)
    xq_r = xq_d.ap().rearrange("(dc p) n -> p dc n", p=P)
    wq_r = wqT_d.ap().rearrange("(dc p) o -> p dc o", p=P)
    wk_r = wkT_d.ap().rearrange("(dc p) o -> p dc o", p=P)
    wv_r = wvT_d.ap().rearrange("(dc p) o -> p dc o", p=P)
    mk_r = mk_d.ap().rearrange("j p q -> p j q")
    out_r = out_d.ap().rearrange("(s p) o -> p s o", p=P)

    with tile.TileContext(nc) as tc:
        with tc.tile_pool(name="pers", bufs=1) as pers:
            qT = pers.tile([P, NOC, NQ_OWN], bf, tag="qT")
            kT = pers.tile([P, NOC, N], bf, tag="kT")
            v = pers.tile([P, NKC, D], bf, tag="v")
            mks = pers.tile([P, 24, QT], bf, tag="masks")
            ones = pers.tile([P, 1], bf, tag="ones")

            nc.vector.memset(ones[:], 1.0)

            # PE pre-warm while the first DMAs land (HAM ramp).
            with tc.tile_pool(name="warm", bufs=1, space="PSUM") as warmps:
                wsrc = pers.tile([P, QT], bf, tag="wsrc")
                nc.vector.memset(wsrc[:], 0.0)
                wps = warmps.tile([P, QT], f32, tag="warm")
                for _ in range(10):
                    nc.tensor.matmul(wps, wsrc[:, :P], wsrc[:], start=True, stop=True)

            # ---- projections (K, then Q, then V) ----
            with (
                tc.tile_pool(name="wp", bufs=1) as wp,
                tc.tile_pool(name="xsp", bufs=2) as xsp,
                tc.tile_pool(name="psA", bufs=4, space="PSUM") as psA,
            ):
                wk = wp.tile([P, NDC, D], bf, tag="wk")
                wq = wp.tile([P, NDC, D], bf, tag="wq")
                wv = wp.tile([P, NDC, D], bf, tag="wv")
                xts = []
                for kt in range(NQT):
                    xts.append(xsp.tile([P, NDC, QT], bf, tag="xs", name=f"xk{kt}"))
                # DMA issue order = consumption order.
                for dc in range(NDC):
                    nc.sync.dma_start(wk[:, dc, :], wk_r[:, dc, :])
                    nc.sync.dma_start(xts[0][:, dc, :], xT_r[:, dc, :QT])
                for kt in range(1, NQT):
                    for dc in range(NDC):
                        sl = slice(kt * QT, (kt + 1) * QT)
                        nc.sync.dma_start(xts[kt][:, dc, :], xT_r[:, dc, sl])
                for dc in range(NDC):
                    nc.sync.dma_start(wq[:, dc, :], wq_r[:, dc, :])
                for dc in range(NDC):
                    nc.sync.dma_start(wv[:, dc, :], wv_r[:, dc, :])
                nc.sync.dma_start(mks[:], mk_r)

                # K projection: kT[o, k] (full 2048 k)
                for kt in range(NQT):
                    for oc in range(NOC):
                        ps = psA.tile([P, QT], f32, tag="proj")
                        for dc in range(NDC):
                            nc.tensor.matmul(
                                ps,
                                wk[:, dc, oc * P : (oc + 1) * P],
                                xts[kt][:, dc, :],
                                start=(dc == 0),
                                stop=(dc == NDC - 1),
                            )
                        nc.vector.tensor_copy(kT[:, oc, kt * QT : (kt + 1) * QT], ps)

                # Q projection: qT[o, q] (own 1024 q)
                for qt in range(2):
                    xqt = xsp.tile([P, NDC, QT], bf, tag="xs", name=f"xq{qt}")
                    for dc in range(NDC):
                        sl = slice(qt * QT, (qt + 1) * QT)
                        nc.sync.dma_start(xqt[:, dc, :], xq_r[:, dc, sl])
                    for oc in range(NOC):
                        ps = psA.tile([P, QT], f32, tag="proj")
                        for dc in range(NDC):
                            nc.tensor.matmul(
                                ps,
                                wq[:, dc, oc * P : (oc + 1) * P],
                                xqt[:, dc, :],
                                start=(dc == 0),
                                stop=(dc == NDC - 1),
                            )
                        nc.vector.tensor_copy(qT[:, oc, qt * QT : (qt + 1) * QT], ps)

                # V projection: v[k, o] (full o)
                for kt in range(NQT):
                    xvt = xsp.tile([P, NDC, QT], bf, tag="xs", name=f"xv{kt}")
                    for dc in range(NDC):
                        sl = slice(kt * QT, (kt + 1) * QT)
                        nc.sync.dma_start(xvt[:, dc, :], xT_r[:, dc, sl])
                    for kl in range(4):
                        kc = kt * 4 + kl
                        for oh in range(2):
                            ps = psA.tile([P, QT], f32, tag="proj")
                            for dc in range(NDC):
                                nc.tensor.matmul(
                                    ps,
                                    xvt[:, dc, kl * P : (kl + 1) * P],
                                    wv[:, dc, oh * QT : (oh + 1) * QT],
                                    start=(dc == 0),
                                    stop=(dc == NDC - 1),
                                )
                            nc.vector.tensor_copy(
                                v[:, kc, oh * QT : (oh + 1) * QT], ps
                            )

            # ---- attention ----
            with (
                tc.tile_pool(name="stps", bufs=2, space="PSUM") as stps,
                tc.tile_pool(name="avps", bufs=2, space="PSUM") as avps,
                tc.tile_pool(name="smps", bufs=2, space="PSUM") as smps,
                tc.tile_pool(name="pTp", bufs=2) as pTp,
                tc.tile_pool(name="outst", bufs=4) as outst,
                tc.tile_pool(name="rcpp", bufs=4) as rcpp,
            ):
                for slot in range(2):
                    nk = SLOT_NKC[slot]
                    sheet = pTp.tile([P, NKC, QT], bf, tag="sheet")
                    for c in range(nk):
                        ps = stps.tile([P, QT], f32, tag="st")
                        for oc in range(NOC):
                            nc.tensor.matmul(
                                ps,
                                kT[:, oc, c * P : (c + 1) * P],
                                qT[:, oc, slot * QT : (slot + 1) * QT],
                                start=(oc == 0),
                                stop=(oc == NOC - 1),
                            )
                        nc.scalar.activation(
                            sheet[:, c, :], ps, Exp, bias=0.0, scale=SCALE
                        )
                        m = slot * 8 + c
                        nc.vector.tensor_mul(
                            sheet[:, c, :], sheet[:, c, :], mks[:, m, :]
                        )
                    for j in range(4):
                        e = AV_ENV[slot][j]
                        av = avps.tile([P, 2, QT], f32, tag="av")
                        sm = smps.tile([P, 1], f32, tag="sm")
                        for c in range(e):
                            psl = sheet[:, c, j * P : (j + 1) * P]
                            nc.tensor.matmul(
                                av[:, 0, :], psl, v[:, c, :QT],
                                start=(c == 0), stop=(c == e - 1),
                            )
                            nc.tensor.matmul(
                                av[:, 1, :], psl, v[:, c, QT:],
                                start=(c == 0), stop=(c == e - 1),
                            )
                            nc.tensor.matmul(
                                sm, psl, ones[:], start=(c == 0), stop=(c == e - 1)
                            )
                        r = rcpp.tile([P, 1], f32, tag="rcp")
                        nc.vector.reciprocal(r[:], sm)
                        ot = outst.tile([P, D], f32, tag="out")
                        nc.vector.tensor_scalar_mul(ot[:, :QT], av[:, 0, :], r[:])
                        nc.vector.tensor_scalar_mul(ot[:, QT:], av[:, 1, :], r[:])
                        nc.sync.dma_start(out_r[:, slot * 4 + j, :], ot[:])

    nc.compile()
    return nc


def _masks_np(role):
    subs = ROLE_SUBTILES[role]
    k = np.arange(P)[:, None]
    q_loc = np.arange(QT)[None, :]
    # original global query index for each local q column, per slot
    qg = []
    for slot in range(2):
        og = np.empty(QT, dtype=np.int64)
        for j in range(4):
            s = subs[slot * 4 + j]
            og[j * P : (j + 1) * P] = s * P + np.arange(P)
        qg.append(og[None, :])
    ms = []
    for c in range(8):
        ms.append(P * c + k <= qg[0])
    for c in range(16):
        ms.append(P * c + k <= qg[1])
    return np.stack(ms).astype(BF16)


def get_module():
    if "nc" not in _CACHE:
        _CACHE["nc"] = _build_module()
    return _CACHE["nc"]


def make_in_maps(x, W_q, W_k, W_v):
    xT = np.ascontiguousarray(
        np.asarray(x, dtype=np.float32).transpose(0, 2, 1)
    ).astype(BF16)
    wqT = np.ascontiguousarray(np.asarray(W_q, dtype=np.float32).T).astype(BF16)
    wkT = np.ascontiguousarray(np.asarray(W_k, dtype=np.float32).T).astype(BF16)
    wvT = np.ascontiguousarray(np.asarray(W_v, dtype=np.float32).T).astype(BF16)
    masks = [_masks_np(r) for r in range(2)]
    in_maps = []
    for c in range(NCORES):
        b, r = c // 2, c % 2
        xq = np.concatenate(
            [xT[b][:, s * P : (s + 1) * P] for s in ROLE_SUBTILES[r]], axis=1
        )
        in_maps.append(
            {
                "xT": xT[b],
                "xTq": np.ascontiguousarray(xq),
                "wqT": wqT,
                "wkT": wkT,
                "wvT": wvT,
                "masks": masks[r],
            }
        )
    return in_maps


def kernel(x, W_q, W_k, W_v):
    from concourse.bass_utils import run_bass_kernel_spmd

    nc = get_module()
    in_maps = make_in_maps(x, W_q, W_k, W_v)
    res = run_bass_kernel_spmd(
        nc,
        in_maps,
        list(range(NCORES)),
        trace=bool(int(os.environ.get("KERNEL_TRACE", "0"))),
    )
    _CACHE["last_result"] = res
    out = np.empty((B, N, D), dtype=np.float32)
    for c in range(NCORES):
        b, r = c // 2, c % 2
        res_out = res.results[c]["out"]
        for i, s in enumerate(ROLE_SUBTILES[r]):
            out[b, s * P : (s + 1) * P, :] = res_out[i * P : (i + 1) * P]
    return out


# revision 14
# speedup vs baseline: 1.2230x; 1.0245x over previous
"""Causal single-head attention (b=4, n=2048, d=1024, fp32) on 8 TRN2 NeuronCores.

Sharding v2 — uniform padded zig-zag q-split. Core c = (batch c//2, role c%2).
Each role owns 8 of the 16 query subtiles of its batch (zig-zag interleaved,
see ROLE_SUBTILES); every core produces out rows for its own 1024 queries
with the FULL 1024 features.

The SPMD program is identical on all cores; the role only changes host-side
data: which columns land in xTq (own queries), the causal masks, and where
host scatters the output rows. Causal work is padded to the per-slot envelope
(slot0: 8 k-chunks, slot1: 16; AV per subtile j: 5+j / 13+j chunks) so both
roles run the same instruction stream; mask data zeroes the padding.

Per core pipeline (all matmuls bf16 -> fp32 PSUM):
  kT[o,k] = W_k x.T (full 2048 k), qT[o,q] = W_q xq.T (own 1024 q),
  v[k,o] = x W_v.T (full o);  sT[k,q] = scores (contraction over o);
  P = exp(sT/32) * mask (no max subtraction; scaled scores are in [-2.6, 2.6]);
  row sums l[q] via ones-matmul; out[q,o] = (P.T-contraction) / l.
"""

import os
import sys

if os.path.isdir("/opt/trn_rl_repo") and "/opt/trn_rl_repo" not in sys.path:
    sys.path.insert(0, "/opt/trn_rl_repo")

import numpy as np
import ml_dtypes

BF16 = ml_dtypes.bfloat16

B, N, D = 4, 2048, 1024
NCORES = 8
P = 128
QT = 512
NQT = N // QT      # 4 orig q tiles
NKC = N // P       # 16 k chunks
NDC = D // P       # 8 d chunks
NOC = D // P       # 8 o chunks
NQ_OWN = 1024      # own queries per core
SCALE = 1.0 / 32.0

# Zig-zag assignment of the 16 query subtiles (128 rows each) to the two
# roles, chosen so the elementwise-max envelope across roles is minimal:
# slot0 = own subtiles drawn from {0..7}, slot1 from {8..15}.
ROLE_SUBTILES = {
    0: (0, 3, 4, 7, 8, 11, 12, 15),
    1: (1, 2, 5, 6, 9, 10, 13, 14),
}
SLOT_NKC = (8, 16)                  # S^T envelope chunks per slot
AV_ENV = ((2, 4, 6, 8), (10, 12, 14, 16))  # AV envelope per (slot, position)

_CACHE = {}


def _build_module():
    from concourse import bacc
    import concourse.tile as tile
    import concourse.mybir as mybir

    bf = mybir.dt.bfloat16
    f32 = mybir.dt.float32
    Exp = mybir.ActivationFunctionType.Exp

    nc = bacc.Bacc("TRN2", target_bir_lowering=False, debug=False, num_devices=NCORES)

    xT_d = nc.dram_tensor("xT", [D, N], bf, kind="ExternalInput")
    xq_d = nc.dram_tensor("xTq", [D, NQ_OWN], bf, kind="ExternalInput")
    wqT_d = nc.dram_tensor("wqT", [D, D], bf, kind="ExternalInput")
    wkT_d = nc.dram_tensor("wkT", [D, D], bf, kind="ExternalInput")
    wvT_d = nc.dram_tensor("wvT", [D, D], bf, kind="ExternalInput")
    mk_d = nc.dram_tensor("masks", [24, P, QT], bf, kind="ExternalInput")
    out_d = nc.dram_tensor("out", [NQ_OWN, D], f32, kind="ExternalOutput")

    xT_r = xT_d.ap().rearrange("(dc p) n -> p dc n", p=P)
    xq_r = xq_d.ap().rearrange("(dc p) n -> p dc n", p=P)
    wq_r = wqT_d.ap().rearrange("(dc p) o -> p dc o", p=P)
    wk_r = wkT_d.ap().rearrange("(dc p) o -> p dc o", p=P)
    wv_r = wvT_d.ap().rearrange("(dc p) o -> p dc o", p=P)
    mk_r = mk_d.ap().rearrange("j p q -> p j q")
    out_r = out_d.ap().rearrange("(s p) o -> p s o", p=P)

    with tile.TileContext(nc) as tc:
        with tc.tile_pool(name="pers", bufs=1) as pers:
            qT = pers.tile([P, NOC, NQ_OWN], bf, tag="qT")
            kT = pers.tile([P, NOC, N], bf, tag="kT")
            v = pers.tile([P, NKC, D], bf, tag="v")
            mks = pers.tile([P, 24, QT], bf, tag="masks")
            ones = pers.tile([P, 1], bf, tag="ones")

            nc.vector.memset(ones[:], 1.0)

            # PE pre-warm while the first DMAs land (HAM ramp).
            with tc.tile_pool(name="warm", bufs=1, space="PSUM") as warmps:
                wsrc = pers.tile([P, QT], bf, tag="wsrc")
                nc.vector.memset(wsrc[:], 0.0)
                wps = warmps.tile([P, QT], f32, tag="warm")
                for _ in range(10):
                    nc.tensor.matmul(wps, wsrc[:, :P], wsrc[:], start=True, stop=True)

            # ---- projections (K, then Q, then V) ----
            with (
                tc.tile_pool(name="wp", bufs=1) as wp,
                tc.tile_pool(name="xsp", bufs=4) as xsp,
                tc.tile_pool(name="psA", bufs=4, space="PSUM") as psA,
            ):
                wk = wp.tile([P, NDC, D], bf, tag="wk")
                wq = wp.tile([P, NDC, D], bf, tag="wq")
                wv = wp.tile([P, NDC, D], bf, tag="wv")
                xts = []
                for kt in range(NQT):
                    xts.append(xsp.tile([P, NDC, QT], bf, tag="xs", name=f"xk{kt}"))
                # DMA issue order = consumption order.
                for dc in range(NDC):
                    nc.sync.dma_start(wk[:, dc, :], wk_r[:, dc, :])
                    nc.sync.dma_start(xts[0][:, dc, :], xT_r[:, dc, :QT])
                for kt in range(1, NQT):
                    for dc in range(NDC):
                        sl = slice(kt * QT, (kt + 1) * QT)
                        nc.sync.dma_start(xts[kt][:, dc, :], xT_r[:, dc, sl])
                for dc in range(NDC):
                    nc.sync.dma_start(wq[:, dc, :], wq_r[:, dc, :])
                for dc in range(NDC):
                    nc.sync.dma_start(wv[:, dc, :], wv_r[:, dc, :])
                nc.sync.dma_start(mks[:], mk_r)

                # K projection: kT[o, k] (full 2048 k)
                for kt in range(NQT):
                    for oc in range(NOC):
                        ps = psA.tile([P, QT], f32, tag="proj")
                        for dc in range(NDC):
                            nc.tensor.matmul(
                                ps,
                                wk[:, dc, oc * P : (oc + 1) * P],
                                xts[kt][:, dc, :],
                                start=(dc == 0),
                                stop=(dc == NDC - 1),
                            )
                        nc.vector.tensor_copy(kT[:, oc, kt * QT : (kt + 1) * QT], ps)

                # Q projection: qT[o, q] (own 1024 q)
                for qt in range(2):
                    xqt = xsp.tile([P, NDC, QT], bf, tag="xs", name=f"xq{qt}")
                    for dc in range(NDC):
                        sl = slice(qt * QT, (qt + 1) * QT)
                        nc.sync.dma_start(xqt[:, dc, :], xq_r[:, dc, sl])
                    for oc in range(NOC):
                        ps = psA.tile([P, QT], f32, tag="proj")
                        for dc in range(NDC):
                            nc.tensor.matmul(
                                ps,
                                wq[:, dc, oc * P : (oc + 1) * P],
                                xqt[:, dc, :],
                                start=(dc == 0),
                                stop=(dc == NDC - 1),
                            )
                        nc.vector.tensor_copy(qT[:, oc, qt * QT : (qt + 1) * QT], ps)

                # V projection: v[k, o] (full o)
                for kt in range(NQT):
                    xvt = xsp.tile([P, NDC, QT], bf, tag="xs", name=f"xv{kt}")
                    for dc in range(NDC):
                        sl = slice(kt * QT, (kt + 1) * QT)
                        nc.sync.dma_start(xvt[:, dc, :], xT_r[:, dc, sl])
                    for kl in range(4):
                        kc = kt * 4 + kl
                        for oh in range(2):
                            ps = psA.tile([P, QT], f32, tag="proj")
                            for dc in range(NDC):
                                nc.tensor.matmul(
                                    ps,
                                    xvt[:, dc, kl * P : (kl + 1) * P],
                                    wv[:, dc, oh * QT : (oh + 1) * QT],
                                    start=(dc == 0),
                                    stop=(dc == NDC - 1),
                                )
                            nc.vector.tensor_copy(
                                v[:, kc, oh * QT : (oh + 1) * QT], ps
                            )

            # ---- attention ----
            with (
                tc.tile_pool(name="stps", bufs=2, space="PSUM") as stps,
                tc.tile_pool(name="avps", bufs=2, space="PSUM") as avps,
                tc.tile_pool(name="smps", bufs=2, space="PSUM") as smps,
                tc.tile_pool(name="pTp", bufs=2) as pTp,
                tc.tile_pool(name="outst", bufs=4) as outst,
                tc.tile_pool(name="rcpp", bufs=4) as rcpp,
            ):
                for slot in range(2):
                    sheet = pTp.tile([P, NKC, QT], bf, tag="sheet")
                    # scores at q-half (256) granularity: each half only needs
                    # chunks up to its own causal envelope (= AV_ENV[slot][2h+1])
                    for h in range(2):
                        nk = AV_ENV[slot][2 * h + 1]
                        hq = slice(h * (QT // 2), (h + 1) * (QT // 2))
                        for c in range(nk):
                            ps = stps.tile([P, QT // 2], f32, tag="st")
                            for oc in range(NOC):
                                nc.tensor.matmul(
                                    ps,
                                    kT[:, oc, c * P : (c + 1) * P],
                                    qT[:, oc, slot * QT + h * (QT // 2) :
                                       slot * QT + (h + 1) * (QT // 2)],
                                    start=(oc == 0),
                                    stop=(oc == NOC - 1),
                                )
                            nc.scalar.activation(
                                sheet[:, c, hq], ps, Exp, bias=0.0, scale=SCALE
                            )
                            m = slot * 8 + c
                            nc.vector.tensor_mul(
                                sheet[:, c, hq], sheet[:, c, hq], mks[:, m, hq]
                            )
                    for j in range(4):
                        e = AV_ENV[slot][j]
                        av = avps.tile([P, 2, QT], f32, tag="av")
                        sm = smps.tile([P, 1], f32, tag="sm")
                        for c in range(e):
                            psl = sheet[:, c, j * P : (j + 1) * P]
                            nc.tensor.matmul(
                                av[:, 0, :], psl, v[:, c, :QT],
                                start=(c == 0), stop=(c == e - 1),
                            )
                            nc.tensor.matmul(
                                av[:, 1, :], psl, v[:, c, QT:],
                                start=(c == 0), stop=(c == e - 1),
                            )
                            nc.tensor.matmul(
                                sm, psl, ones[:], start=(c == 0), stop=(c == e - 1)
                            )
                        r = rcpp.tile([P, 1], f32, tag="rcp")
                        nc.vector.reciprocal(r[:], sm)
                        ot = outst.tile([P, D], f32, tag="out")
                        nc.vector.tensor_scalar_mul(ot[:, :QT], av[:, 0, :], r[:])
                        nc.vector.tensor_scalar_mul(ot[:, QT:], av[:, 1, :], r[:])
                        nc.sync.dma_start(out_r[:, slot * 4 + j, :], ot[:])

    nc.compile()
    return nc


def _masks_np(role):
    subs = ROLE_SUBTILES[role]
    k = np.arange(P)[:, None]
    q_loc = np.arange(QT)[None, :]
    # original global query index for each local q column, per slot
    qg = []
    for slot in range(2):
        og = np.empty(QT, dtype=np.int64)
        for j in range(4):
            s = subs[slot * 4 + j]
            og[j * P : (j + 1) * P] = s * P + np.arange(P)
        qg.append(og[None, :])
    ms = []
    for c in range(8):
        ms.append(P * c + k <= qg[0])
    for c in range(16):
        ms.append(P * c + k <= qg[1])
    return np.stack(ms).astype(BF16)


def get_module():
    if "nc" not in _CACHE:
        _CACHE["nc"] = _build_module()
    return _CACHE["nc"]


def make_in_maps(x, W_q, W_k, W_v):
    xT = np.ascontiguousarray(
        np.asarray(x, dtype=np.float32).transpose(0, 2, 1)
    ).astype(BF16)
    wqT = np.ascontiguousarray(np.asarray(W_q, dtype=np.float32).T).astype(BF16)
    wkT = np.ascontiguousarray(np.asarray(W_k, dtype=np.float32).T).astype(BF16)
    wvT = np.ascontiguousarray(np.asarray(W_v, dtype=np.float32).T).astype(BF16)
    masks = [_masks_np(r) for r in range(2)]
    in_maps = []
    for c in range(NCORES):
        b, r = c // 2, c % 2
        xq = np.concatenate(
            [xT[b][:, s * P : (s + 1) * P] for s in ROLE_SUBTILES[r]], axis=1
        )
        in_maps.append(
            {
                "xT": xT[b],
                "xTq": np.ascontiguousarray(xq),
                "wqT": wqT,
                "wkT": wkT,
                "wvT": wvT,
                "masks": masks[r],
            }
        )
    return in_maps


def kernel(x, W_q, W_k, W_v):
    from concourse.bass_utils import run_bass_kernel_spmd

    nc = get_module()
    in_maps = make_in_maps(x, W_q, W_k, W_v)
    res = run_bass_kernel_spmd(
        nc,
        in_maps,
        list(range(NCORES)),
        trace=bool(int(os.environ.get("KERNEL_TRACE", "0"))),
    )
    _CACHE["last_result"] = res
    out = np.empty((B, N, D), dtype=np.float32)
    for c in range(NCORES):
        b, r = c // 2, c % 2
        res_out = res.results[c]["out"]
        for i, s in enumerate(ROLE_SUBTILES[r]):
            out[b, s * P : (s + 1) * P, :] = res_out[i * P : (i + 1) * P]
    return out
